# revision 4
# baseline (speedup 1.0000x reference)
"""GCN (3-layer, JK-concat) Trainium2 kernel, 8-core SPMD.

Src-sharded + ReduceScatter design:
 - Nodes split into halves A/B x 8 cores (640 padded rows per (core, half);
   global table row = 5120h + 640c + j); each core owns 1280 local rows.
 - Per layer: local GEMM t' = dinv_src * (h @ W') -> fp8 hi/lo quantize
   (hi = fp8(t'), lo = fp8(t' - hi): bf16-level accuracy at fp8 DoubleRow
   matmul rate) -> per global dst tile, DoubleRow matmuls with one-hot sel
   matrices as stationary weights scatter-accumulate edge messages in PSUM
   -> bf16 partial table in DRAM -> per-half ReduceScatter(add) -> each
   core gets its aggregated rows -> relu(dinv_dst * agg + c) -> transpose
   for the next GEMM.  BN folded into W' and c.
 - Software pipeline: pass1 (src pairs 0-1 -> SBUF stash) fills the
   RS_B(l-1) window; pass2 (pairs 2-4 + stash merge via PE identity matmul
   or DVE add) feeds RS_A(l); full-sweep B-dst tiles fill the RS_A(l)
   window.  Engine placement: Pool = collectives + agg loads (fire in
   stream order right as each RS completes), DVE = relu/quantize/merges,
   ACT = evicts/transpose-copies (no DMAs: engine compute is FIFO-ordered
   behind its own DMA completions), SP = batched partial writes + inputs.
 - JK: out = sum_l hT_l.T @ lin_w_l (+ lin_b), fp32, local, split per half.
"""
import sys
sys.path.insert(0, "/opt/trn_rl_repo")
import numpy as np
import ml_dtypes

import concourse.bass as bass
import concourse.bacc as bacc
import concourse.mybir as mybir
import concourse.tile as tile
from concourse.bass_utils import run_bass_kernel_spmd

N = 10000
D = 256
L = 3
BN_EPS = 1e-5
NCORES = 8
# even halves: A = dst tiles 0-39, B = 40-79 (src chunks 0-4 / 5-9)
CPH_A, CPH_B = 625, 625       # real nodes per core per half
PADH_A, PADH_B = 640, 640     # padded rows per core per half
PPC = PADH_A + PADH_B         # local rows per core (1280)
NT = PPC // 128               # local row tiles (10)
NTA = PADH_A // 128           # local A tiles (4)
ROWS_A = NCORES * PADH_A      # 4096 global A rows
ROWS_B = NCORES * PADH_B      # 6144 global B rows
GTA = ROWS_A // 128           # 32 global A dst tiles
GT = (ROWS_A + ROWS_B) // 128  # 80 global dst tiles
GTB = GT - GTA                # 48 global B dst tiles
PAIRS = NT // 2               # 5 local src chunk-pairs
BF16 = ml_dtypes.bfloat16
F8 = ml_dtypes.float8_e4m3

_TRACE = False


# ----------------------------------------------------------------- host prep
def _preprocess(x, edge_index, Ws, bs, bn_scale, bn_bias, bn_mean, bn_var,
                lin_w, lin_b):
    src = np.asarray(edge_index[0], np.int64)
    dst = np.asarray(edge_index[1], np.int64)
    loops = np.arange(N, dtype=np.int64)
    src_f = np.concatenate([src, loops])
    dst_f = np.concatenate([dst, loops])

    deg = np.bincount(dst_f, minlength=N).astype(np.float64)
    dinv = np.where(deg > 0, 1.0 / np.sqrt(deg), 0.0).astype(np.float32)

    # node n -> (owner core, local row, global table row); first 8*CPH_A
    # nodes go to half A, rest to half B
    n = np.arange(N)
    NA = NCORES * CPH_A
    in_a = n < NA
    ia = n
    ib = n - NA
    owner = np.where(in_a, ia // CPH_A, ib // CPH_B)
    local = np.where(in_a, ia % CPH_A, PADH_A + ib % CPH_B)
    grow = np.where(in_a, PADH_A * owner + ia % CPH_A,
                    ROWS_A + PADH_B * owner + ib % CPH_B)

    # per-core xT and dinv tiles
    xs = np.asarray(x, np.float32)
    xT = np.zeros((NCORES, D, PPC), np.float32)
    xT[owner, :, local] = xs
    dinv_t = np.zeros((NCORES, 128, NT), np.float32)
    dinv_t[owner, local % 128, local // 128] = dinv

    # sel blocks: [core, src_part, dst_tile, pair, half_of_pair, dst_part]
    oc = owner[src_f]
    sl = local[src_f]
    dr = grow[dst_f]
    sel = np.zeros((NCORES, 128, GT, PAIRS, 2, 128), np.uint8)
    np.add.at(sel, (oc, sl % 128, dr // 128, sl // 256, (sl // 128) % 2,
                    dr % 128), 1)
    assert sel.max() < 16

    # BN folding
    rs = 1.0 / np.sqrt(np.asarray(bn_var, np.float64) + BN_EPS)
    colscale = rs * np.asarray(bn_scale, np.float64)           # [L,D]
    Wp = np.asarray(Ws, np.float64) * colscale[:, None, :]     # [L,D,D]
    c = ((np.asarray(bs, np.float64) - np.asarray(bn_mean, np.float64))
         * colscale + np.asarray(bn_bias, np.float64))         # [L,D]

    return dict(
        sel=sel, xT=xT, dinv_t=dinv_t,
        Wp=Wp.astype(np.float32), c=c.astype(np.float32),
        lin_w=np.asarray(lin_w, np.float32), lin_b=np.asarray(lin_b, np.float32),
        owner=owner, local=local,
    )


# -------------------------------------------------------------- device build
def _build(Wp, c, lin_w, lin_b):
    c_zero = bool(np.all(np.abs(c) < 1e-12))
    b_zero = bool(np.all(np.abs(lin_b) < 1e-12))

    nc = bacc.Bacc("TRN2", target_bir_lowering=False, debug=False,
                   enable_asserts=True, num_devices=NCORES)
    xT_d = nc.dram_tensor("xT", [D, PPC], mybir.dt.float32, kind="ExternalInput")
    sel_d = nc.dram_tensor("sel", [128, GT * PAIRS * 2 * 128], mybir.dt.float8e4,
                           kind="ExternalInput")
    dinv_d = nc.dram_tensor("dinv", [128, NT], mybir.dt.float32,
                            kind="ExternalInput")
    out_d = nc.dram_tensor("out", [PPC, D], mybir.dt.float32,
                           kind="ExternalOutput")

    ident_d = nc.inline_tensor(np.eye(128, dtype=BF16), name="ident")
    Wp_d = nc.inline_tensor(Wp.astype(BF16), name="Wp")            # [L,D,D]
    linw_d = nc.inline_tensor(lin_w.astype(BF16), name="linw")     # [768,D]
    if not c_zero:
        c_d = nc.inline_tensor(
            np.broadcast_to(c[:, None, :], (L, 128, D)).copy(), name="cvec")
    if not b_zero:
        linb_d = nc.inline_tensor(
            np.broadcast_to(lin_b[None, :], (128, D)).copy(), name="linb")

    DR = mybir.MatmulPerfMode.DoubleRow

    with tile.TileContext(nc) as tc:
        with (
            tc.tile_pool(name="const", bufs=1) as constp,
            tc.tile_pool(name="hT", bufs=1) as hTp,
            tc.tile_pool(name="plane", bufs=2) as planep,
            tc.tile_pool(name="work", bufs=3) as workp,
            tc.tile_pool(name="part", bufs=2) as partp,
            tc.tile_pool(name="hb", bufs=10) as hbp,
            tc.tile_pool(name="dram", bufs=2, space="DRAM") as dramp,
            tc.tile_pool(name="psA", bufs=3, space="PSUM") as psA,
            tc.tile_pool(name="psB", bufs=2, space="PSUM") as psB,
            tc.tile_pool(name="psT", bufs=2, space="PSUM") as psT,
        ):
            # ---- x^T load first (it gates layer-0 GEMM), SP queue
            hT = {}
            xfs = []
            for f in range(2):
                xf = workp.tile([128, PPC], mybir.dt.float32, tag="xTf32")
                nc.sync.dma_start(xf[:], xT_d[128 * f:128 * (f + 1), :])
                xfs.append(xf)
            # ---- critical constants (SP queue, right after xT)
            ident = constp.tile([128, 128], mybir.dt.bfloat16, tag="ident")
            nc.sync.dma_start(ident[:], ident_d[:])
            dinv_sb = constp.tile([128, NT], mybir.dt.float32, tag="dinv")
            nc.sync.dma_start(dinv_sb[:], dinv_d[:])
            W_sb = constp.tile([128, L * 2, D], mybir.dt.bfloat16, tag="W")
            nc.sync.dma_start(
                W_sb[:], Wp_d.ap().rearrange("l (h p) d -> p (l h) d", p=128))
            # sel matrices: pieces 0-3 (dst tiles 0-39) on SP right behind the
            # weights; pieces 4-7 on the Pool queue (Pool is otherwise unused
            # until the first collective, which fires after tile 39).
            # No DMAs on ACT: an engine's compute is FIFO-ordered behind its
            # own DMA completions.
            sel_sb = constp.tile([128, GT, PAIRS, 2, 128], mybir.dt.float8e4,
                                 tag="sel")
            PIECE = 10 * PAIRS * 2 * 128
            for k in range(8):
                qeng = nc.sync if k < 4 else nc.gpsimd
                qeng.dma_start(
                    sel_sb[:, 10 * k:10 * (k + 1), :, :, :],
                    sel_d[:, PIECE * k:PIECE * (k + 1)].rearrange(
                        "p (t q i d) -> p t q i d", t=10, q=PAIRS, i=2))
            # non-critical constants after the sel pieces
            linw_sb = constp.tile([128, L * 2, D], mybir.dt.bfloat16, tag="linw")
            nc.sync.dma_start(
                linw_sb[:], linw_d.ap().rearrange("(k p) d -> p k d", p=128))
            if not c_zero:
                c_sb = constp.tile([128, L, D], mybir.dt.float32, tag="cvec")
                nc.sync.dma_start(c_sb[:], c_d.ap().rearrange("l p d -> p l d"))
            if not b_zero:
                linb_sb = constp.tile([128, D], mybir.dt.float32, tag="linb")
                nc.sync.dma_start(linb_sb[:], linb_d.ap())

            # ---- cast x -> hT_cur
            for f in range(2):
                t = hTp.tile([128, PPC], mybir.dt.bfloat16, tag=f"hT_x_{f}")
                nc.vector.tensor_copy(t[:], xfs[f][:])
                hT[f] = t

            # stash for pass-1 partial sums (pairs 0-1, A-dst tiles only)
            stash = constp.tile([128, GTA, D], mybir.dt.bfloat16, tag="stash")

            def gemm_quant(l, bs, hT_in, hi_pl, lo_pl):
                """GEMM + dinv pre-scale + fp8 hi/lo quantize for row tiles bs."""
                for b in bs:
                    tp = psB.tile([128, D], mybir.dt.float32, tag="gemm",
                                  name=f"tp_{l}_{b}")
                    for f in range(2):
                        nc.tensor.matmul(
                            tp[:], hT_in[f][:, 128 * b:128 * (b + 1)],
                            W_sb[:, 2 * l + f, :],
                            start=(f == 0), stop=(f == 1))
                    tb = workp.tile([128, D], mybir.dt.float32, tag="tb",
                                    name=f"tb_{l}_{b}")
                    nc.vector.tensor_scalar(
                        tb[:], tp[:], dinv_sb[:, b:b + 1], None,
                        mybir.AluOpType.mult)
                    hi = hi_pl[:, b // 2, b % 2, :]
                    nc.scalar.activation(hi, tb[:],
                                         mybir.ActivationFunctionType.Copy)
                    nc.vector.tensor_tensor(
                        lo_pl[:, b // 2, b % 2, :], tb[:], hi,
                        mybir.AluOpType.subtract)

            def consume_pre(l, h, agg, hbs):
                """agg loads on Pool (fires right as RS completes in its
                stream); relu(dinv*agg (+c)) on DVE."""
                nth = NTA if h == 0 else NT - NTA
                for k in range(nth):
                    b = NTA * h + k
                    asb = workp.tile([128, D], mybir.dt.bfloat16, tag="asb",
                                     name=f"asb_{l}_{b}")
                    nc.gpsimd.dma_start(asb[:],
                                        agg[h][128 * k:128 * (k + 1), :])
                    hb = hbp.tile([128, D], mybir.dt.bfloat16, tag="hb",
                                  name=f"hb_{l}_{b}")
                    if c_zero:
                        nc.vector.tensor_scalar(
                            hb[:], asb[:], dinv_sb[:, b:b + 1], 0.0,
                            mybir.AluOpType.mult, mybir.AluOpType.max)
                    else:
                        tmp = workp.tile([128, D], mybir.dt.float32,
                                         tag="tmp", name=f"tmp_{l}_{b}")
                        nc.vector.tensor_scalar(
                            tmp[:], asb[:], dinv_sb[:, b:b + 1], None,
                            mybir.AluOpType.mult)
                        nc.vector.tensor_tensor(
                            tmp[:], tmp[:], c_sb[:, l, :],
                            mybir.AluOpType.add)
                        nc.vector.tensor_scalar(
                            hb[:], tmp[:], 0.0, None, mybir.AluOpType.max)
                    hbs[b] = hb

            def consume_post(l, h, hbs, new_hT):
                """transpose h tiles into hT (PE + ACT copies)."""
                nth = NTA if h == 0 else NT - NTA
                for k in range(nth):
                    b = NTA * h + k
                    hb = hbs[b]
                    for f in range(2):
                        tps = psT.tile([128, 128], mybir.dt.float32, tag="tr",
                                       name=f"tr_{l}_{b}_{f}")
                        nc.tensor.matmul(tps[:],
                                         hb[:, 128 * f:128 * (f + 1)],
                                         ident[:], start=True, stop=True)
                        nc.scalar.activation(
                            new_hT[f][:, 128 * b:128 * (b + 1)], tps[:],
                            mybir.ActivationFunctionType.Copy)

            # software pipeline per layer l:
            #   consume-A(l-1); GEMM-A(l); pass1(l): A-dst tiles, src pairs
            #     0-1 -> stash            [runs during RS_B(l-1)]
            #   consume-B(l-1); GEMM-B(l); pass2(l): A-dst tiles, src pairs
            #     2-4 + stash merge -> partial A; RS_A(l)
            #   fullB(l): B-dst tiles, all 10 matmuls -> partial B
            #     [runs during RS_A(l)]; RS_B(l)
            P1 = (0, 1)
            P2 = (2, 3, 4)
            GRP = 4
            hT_layers = []
            prev = None      # (agg tiles, new_hT dict) pending consume
            for l in range(L):
                hi_pl = planep.tile([128, PAIRS, 2, D], mybir.dt.float8e4,
                                    tag="hi", name=f"hi_{l}")
                lo_pl = planep.tile([128, PAIRS, 2, D], mybir.dt.float8e4,
                                    tag="lo", name=f"lo_{l}")
                if prev is not None:
                    consume_post(l - 1, 0, prev[0], prev[1])
                    hT = prev[1]
                gemm_quant(l, range(NTA), hT, hi_pl, lo_pl)
                # pass 1: A-dst tiles, src pairs 0-1 -> stash (bf16)
                for t in range(GTA):
                    ps = psA.tile([128, D], mybir.dt.float32, tag="agg",
                                  name=f"ps1_{l}_{t}")
                    for p in P1:
                        nc.tensor.matmul(ps[:], sel_sb[:, t, p, :, :],
                                         hi_pl[:, p, :, :], perf_mode=DR,
                                         start=(p == P1[0]), stop=False)
                        nc.tensor.matmul(ps[:], sel_sb[:, t, p, :, :],
                                         lo_pl[:, p, :, :], perf_mode=DR,
                                         start=False, stop=(p == P1[-1]))
                    if t % 2 == 0:
                        nc.scalar.activation(
                            stash[:, t, :], ps[:],
                            mybir.ActivationFunctionType.Copy)
                    else:
                        nc.vector.tensor_copy(stash[:, t, :], ps[:])
                if prev is not None:
                    consume_post(l - 1, 1, prev[0], prev[1])
                gemm_quant(l, range(NTA, NT), hT, hi_pl, lo_pl)
                partial = [dramp.tile([rows, D], mybir.dt.bfloat16,
                                      tag=f"part{h}", name=f"partial_{l}_{h}")
                           for h, rows in ((0, ROWS_A), (1, ROWS_B))]
                agg = [dramp.tile([rows, D], mybir.dt.bfloat16,
                                  tag=f"agg{h}", name=f"agg_{l}_{h}")
                       for h, rows in ((0, PADH_A), (1, PADH_B))]
                # pass 2: A-dst tiles, src pairs 2-4 + stash merge
                pg = None
                for t in range(GTA):
                    ps = psA.tile([128, D], mybir.dt.float32, tag="agg",
                                  name=f"ps2_{l}_{t}")
                    merge_pe = (t % 2 == 0)
                    for p in P2:
                        nc.tensor.matmul(ps[:], sel_sb[:, t, p, :, :],
                                         hi_pl[:, p, :, :], perf_mode=DR,
                                         start=(p == P2[0]), stop=False)
                        nc.tensor.matmul(ps[:], sel_sb[:, t, p, :, :],
                                         lo_pl[:, p, :, :], perf_mode=DR,
                                         start=False,
                                         stop=(not merge_pe and p == P2[-1]))
                    if merge_pe:
                        # merge the pass-1 stash on the PE: psum += I.T @ stash
                        nc.tensor.matmul(ps[:], ident[:], stash[:, t, :],
                                         start=False, stop=True)
                    if t % GRP == 0:
                        pg = partp.tile([128, GRP, D], mybir.dt.bfloat16,
                                        tag="pg", name=f"pgA_{l}_{t}")
                    slot = pg[:, t % GRP, :]
                    if merge_pe:
                        # plain evict on ACT (gpsimd may not touch PSUM on HW)
                        nc.scalar.activation(
                            slot, ps[:], mybir.ActivationFunctionType.Copy)
                    else:
                        # merge on DVE while evicting
                        nc.vector.tensor_tensor(slot, ps[:], stash[:, t, :],
                                                mybir.AluOpType.add)
                    if t % GRP == GRP - 1:
                        th = t - GRP + 1
                        nc.sync.dma_start(
                            partial[0][128 * th:128 * (th + GRP), :]
                            .rearrange("(g p) d -> p g d", p=128),
                            pg[:])
                nc.gpsimd.collective_compute(
                    "ReduceScatter", mybir.AluOpType.add,
                    replica_groups=[list(range(NCORES))],
                    ins=[partial[0].opt()], outs=[agg[0].opt()])
                hbs = {}
                consume_pre(l, 0, agg, hbs)
                # B-dst tiles: full 10-matmul sweep (fills the RS_A window)
                for tt in range(GTB):
                    t = GTA + tt
                    ps = psA.tile([128, D], mybir.dt.float32, tag="agg",
                                  name=f"psB_{l}_{t}")
                    for p in range(PAIRS):
                        nc.tensor.matmul(ps[:], sel_sb[:, t, p, :, :],
                                         hi_pl[:, p, :, :], perf_mode=DR,
                                         start=(p == 0), stop=False)
                    for p in range(PAIRS):
                        nc.tensor.matmul(ps[:], sel_sb[:, t, p, :, :],
                                         lo_pl[:, p, :, :], perf_mode=DR,
                                         start=False, stop=(p == PAIRS - 1))
                    if tt % GRP == 0:
                        pg = partp.tile([128, GRP, D], mybir.dt.bfloat16,
                                        tag="pg", name=f"pgB_{l}_{t}")
                    slot = pg[:, tt % GRP, :]
                    nc.scalar.activation(
                        slot, ps[:], mybir.ActivationFunctionType.Copy)
                    if tt % GRP == GRP - 1:
                        th = tt - GRP + 1
                        nc.sync.dma_start(
                            partial[1][128 * th:128 * (th + GRP), :]
                            .rearrange("(g p) d -> p g d", p=128),
                            pg[:])
                nc.gpsimd.collective_compute(
                    "ReduceScatter", mybir.AluOpType.add,
                    replica_groups=[list(range(NCORES))],
                    ins=[partial[1].opt()], outs=[agg[1].opt()])
                consume_pre(l, 1, agg, hbs)
                new_hT = {}
                for f in range(2):
                    new_hT[f] = hTp.tile([128, PPC], mybir.dt.bfloat16,
                                         tag=f"hT_{l}_{f}",
                                         name=f"hT_{l}_{f}")
                hT_layers.append(new_hT)
                prev = (hbs, new_hT)
            # final consumes + JK split per half
            consume_post(L - 1, 0, prev[0], prev[1])
            jk_tiles = list(range(NTA))
            for b in jk_tiles:
                op = psB.tile([128, D], mybir.dt.float32, tag="gemm",
                              name=f"jk_{b}")
                kk = 0
                for l in range(L):
                    for f in range(2):
                        nc.tensor.matmul(
                            op[:], hT_layers[l][f][:, 128 * b:128 * (b + 1)],
                            linw_sb[:, 2 * l + f, :],
                            start=(kk == 0), stop=(kk == 5))
                        kk += 1
                ob = workp.tile([128, D], mybir.dt.float32, tag="ob",
                                name=f"ob_{b}")
                if b_zero:
                    nc.vector.tensor_copy(ob[:], op[:])
                else:
                    nc.vector.tensor_tensor(ob[:], op[:], linb_sb[:],
                                            mybir.AluOpType.add)
                nc.sync.dma_start(out_d[128 * b:128 * (b + 1), :], ob[:])
            consume_post(L - 1, 1, prev[0], prev[1])

            # ---- JK final linear, B half
            for b in range(NTA, NT):
                op = psB.tile([128, D], mybir.dt.float32, tag="gemm")
                kk = 0
                for l in range(L):
                    for f in range(2):
                        nc.tensor.matmul(
                            op[:], hT_layers[l][f][:, 128 * b:128 * (b + 1)],
                            linw_sb[:, 2 * l + f, :],
                            start=(kk == 0), stop=(kk == 5))
                        kk += 1
                ob = workp.tile([128, D], mybir.dt.float32, tag="ob")
                if b_zero:
                    nc.vector.tensor_copy(ob[:], op[:])
                else:
                    nc.vector.tensor_tensor(ob[:], op[:], linb_sb[:],
                                            mybir.AluOpType.add)
                nc.sync.dma_start(out_d[128 * b:128 * (b + 1), :], ob[:])
    nc.compile()
    return nc


# ------------------------------------------------------------------- runner
def _build_args(pre):
    return (pre["Wp"], pre["c"], pre["lin_w"], pre["lin_b"])


def _in_maps(pre):
    sel8 = pre["sel"].astype(np.float32).astype(F8)
    return [
        {"xT": np.ascontiguousarray(pre["xT"][cc]),
         "sel": np.ascontiguousarray(
             sel8[cc].reshape(128, GT * PAIRS * 2 * 128)),
         "dinv": np.ascontiguousarray(pre["dinv_t"][cc])}
        for cc in range(NCORES)
    ]


def _postprocess(pre, results):
    out = np.empty((N, D), np.float32)
    owner, local = pre["owner"], pre["local"]
    for cc in range(NCORES):
        m = owner == cc
        out[m] = results[cc]["out"][local[m]]
    return out


def kernel(x, edge_index, Ws, bs, bn_scale, bn_bias, bn_mean, bn_var,
           lin_w, lin_b):
    pre = _preprocess(x, edge_index, Ws, bs, bn_scale, bn_bias, bn_mean,
                      bn_var, lin_w, lin_b)
    nc = _build(*_build_args(pre))
    in_maps = _in_maps(pre)
    kw = {}
    if _TRACE:
        kw = dict(trace=True)
    res = run_bass_kernel_spmd(nc, in_maps, core_ids=list(range(NCORES)), **kw)
    kernel.last_results = res
    return _postprocess(pre, res.results)


# revision 5
# speedup vs baseline: 1.0429x; 1.0429x over previous
"""GCN (3-layer, JK-concat) Trainium2 kernel, 8-core SPMD.

Src-sharded + ReduceScatter design:
 - Nodes split into halves A/B x 8 cores (640 padded rows per (core, half);
   global table row = 5120h + 640c + j); each core owns 1280 local rows.
 - Per layer: local GEMM t' = dinv_src * (h @ W') -> fp8 hi/lo quantize
   (hi = fp8(t'), lo = fp8(t' - hi): bf16-level accuracy at fp8 DoubleRow
   matmul rate) -> per global dst tile, DoubleRow matmuls with one-hot sel
   matrices as stationary weights scatter-accumulate edge messages in PSUM
   -> bf16 partial table in DRAM -> per-half ReduceScatter(add) -> each
   core gets its aggregated rows -> relu(dinv_dst * agg + c) -> transpose
   for the next GEMM.  BN folded into W' and c.
 - Software pipeline: pass1 (src pairs 0-1 -> SBUF stash) fills the
   RS_B(l-1) window; pass2 (pairs 2-4 + stash merge via PE identity matmul
   or DVE add) feeds RS_A(l); full-sweep B-dst tiles fill the RS_A(l)
   window.  Layer 0 (no RS in flight) runs plain full sweeps.
 - Engine placement: Pool = collectives + agg loads (fire in stream order
   right as each RS completes) + most of the sel upload, DVE =
   relu/quantize/merges, ACT = evicts/transpose-copies (no DMAs: engine
   compute is FIFO-ordered behind its own DMA completions), SP = batched
   partial writes + xT/weights/outputs.
 - JK: out = sum_l hT_l.T @ lin_w_l (+ lin_b), fp32, local, split per half.
"""
import sys
sys.path.insert(0, "/opt/trn_rl_repo")
import numpy as np
import ml_dtypes

import concourse.bass as bass
import concourse.bacc as bacc
import concourse.mybir as mybir
import concourse.tile as tile
from concourse.bass_utils import run_bass_kernel_spmd

N = 10000
D = 256
L = 3
BN_EPS = 1e-5
NCORES = 8
# even halves: A = dst tiles 0-39, B = 40-79 (src chunks 0-4 / 5-9)
CPH_A, CPH_B = 625, 625       # real nodes per core per half
PADH_A, PADH_B = 640, 640     # padded rows per core per half
PPC = PADH_A + PADH_B         # local rows per core (1280)
NT = PPC // 128               # local row tiles (10)
NTA = PADH_A // 128           # local A tiles (4)
ROWS_A = NCORES * PADH_A      # 4096 global A rows
ROWS_B = NCORES * PADH_B      # 6144 global B rows
GTA = ROWS_A // 128           # 32 global A dst tiles
GT = (ROWS_A + ROWS_B) // 128  # 80 global dst tiles
GTB = GT - GTA                # 48 global B dst tiles
PAIRS = NT // 2               # 5 local src chunk-pairs
BF16 = ml_dtypes.bfloat16
F8 = ml_dtypes.float8_e4m3

_TRACE = False


# ----------------------------------------------------------------- host prep
def _preprocess(x, edge_index, Ws, bs, bn_scale, bn_bias, bn_mean, bn_var,
                lin_w, lin_b):
    src = np.asarray(edge_index[0], np.int64)
    dst = np.asarray(edge_index[1], np.int64)
    loops = np.arange(N, dtype=np.int64)
    src_f = np.concatenate([src, loops])
    dst_f = np.concatenate([dst, loops])

    deg = np.bincount(dst_f, minlength=N).astype(np.float64)
    dinv = np.where(deg > 0, 1.0 / np.sqrt(deg), 0.0).astype(np.float32)

    # node n -> (owner core, local row, global table row); first 8*CPH_A
    # nodes go to half A, rest to half B
    n = np.arange(N)
    NA = NCORES * CPH_A
    in_a = n < NA
    ia = n
    ib = n - NA
    owner = np.where(in_a, ia // CPH_A, ib // CPH_B)
    local = np.where(in_a, ia % CPH_A, PADH_A + ib % CPH_B)
    grow = np.where(in_a, PADH_A * owner + ia % CPH_A,
                    ROWS_A + PADH_B * owner + ib % CPH_B)

    # per-core xT and dinv tiles
    xs = np.asarray(x, np.float32)
    xT = np.zeros((NCORES, D, PPC), np.float32)
    xT[owner, :, local] = xs
    dinv_t = np.zeros((NCORES, 128, NT), np.float32)
    dinv_t[owner, local % 128, local // 128] = dinv

    # sel blocks: [core, src_part, dst_tile, pair, half_of_pair, dst_part]
    oc = owner[src_f]
    sl = local[src_f]
    dr = grow[dst_f]
    sel = np.zeros((NCORES, 128, GT, PAIRS, 2, 128), np.uint8)
    np.add.at(sel, (oc, sl % 128, dr // 128, sl // 256, (sl // 128) % 2,
                    dr % 128), 1)
    assert sel.max() < 16

    # BN folding
    rs = 1.0 / np.sqrt(np.asarray(bn_var, np.float64) + BN_EPS)
    colscale = rs * np.asarray(bn_scale, np.float64)           # [L,D]
    Wp = np.asarray(Ws, np.float64) * colscale[:, None, :]     # [L,D,D]
    c = ((np.asarray(bs, np.float64) - np.asarray(bn_mean, np.float64))
         * colscale + np.asarray(bn_bias, np.float64))         # [L,D]

    return dict(
        sel=sel, xT=xT, dinv_t=dinv_t,
        Wp=Wp.astype(np.float32), c=c.astype(np.float32),
        lin_w=np.asarray(lin_w, np.float32), lin_b=np.asarray(lin_b, np.float32),
        owner=owner, local=local,
    )


# -------------------------------------------------------------- device build
def _build(Wp, c, lin_w, lin_b):
    c_zero = bool(np.all(np.abs(c) < 1e-12))
    b_zero = bool(np.all(np.abs(lin_b) < 1e-12))

    nc = bacc.Bacc("TRN2", target_bir_lowering=False, debug=False,
                   enable_asserts=True, num_devices=NCORES)
    xT_d = nc.dram_tensor("xT", [D, PPC], mybir.dt.bfloat16,
                          kind="ExternalInput")
    sel_d = nc.dram_tensor("sel", [128, GT * PAIRS * 2 * 128], mybir.dt.float8e4,
                           kind="ExternalInput")
    dinv_d = nc.dram_tensor("dinv", [128, NT], mybir.dt.float32,
                            kind="ExternalInput")
    out_d = nc.dram_tensor("out", [PPC, D], mybir.dt.float32,
                           kind="ExternalOutput")

    ident_d = nc.inline_tensor(np.eye(128, dtype=BF16), name="ident")
    Wp_d = nc.inline_tensor(Wp.astype(BF16), name="Wp")            # [L,D,D]
    linw_d = nc.inline_tensor(lin_w.astype(BF16), name="linw")     # [768,D]
    if not c_zero:
        c_d = nc.inline_tensor(
            np.broadcast_to(c[:, None, :], (L, 128, D)).copy(), name="cvec")
    if not b_zero:
        linb_d = nc.inline_tensor(
            np.broadcast_to(lin_b[None, :], (128, D)).copy(), name="linb")

    DR = mybir.MatmulPerfMode.DoubleRow

    with tile.TileContext(nc) as tc:
        with (
            tc.tile_pool(name="const", bufs=1) as constp,
            tc.tile_pool(name="hT", bufs=1) as hTp,
            tc.tile_pool(name="plane", bufs=2) as planep,
            tc.tile_pool(name="work", bufs=3) as workp,
            tc.tile_pool(name="part", bufs=2) as partp,
            tc.tile_pool(name="hb", bufs=10) as hbp,
            tc.tile_pool(name="dram", bufs=2, space="DRAM") as dramp,
            tc.tile_pool(name="psA", bufs=3, space="PSUM") as psA,
            tc.tile_pool(name="psB", bufs=2, space="PSUM") as psB,
            tc.tile_pool(name="psT", bufs=2, space="PSUM") as psT,
        ):
            # ---- x^T load first (it gates layer-0 GEMM), SP queue; host
            # pre-casts to bf16 so it loads straight into hT tiles
            hT = {}
            for f in range(2):
                t = hTp.tile([128, PPC], mybir.dt.bfloat16, tag=f"hT_x_{f}",
                             name=f"hT_x_{f}")
                nc.sync.dma_start(t[:], xT_d[128 * f:128 * (f + 1), :])
                hT[f] = t
            # ---- critical constants (SP queue, right after xT)
            ident = constp.tile([128, 128], mybir.dt.bfloat16, tag="ident")
            nc.sync.dma_start(ident[:], ident_d[:])
            dinv_sb = constp.tile([128, NT], mybir.dt.float32, tag="dinv")
            nc.sync.dma_start(dinv_sb[:], dinv_d[:])
            W_sb = constp.tile([128, L * 2, D], mybir.dt.bfloat16, tag="W")
            nc.sync.dma_start(
                W_sb[:], Wp_d.ap().rearrange("l (h p) d -> p (l h) d", p=128))
            # sel matrices: pieces 0-3 (dst tiles 0-39) on SP right behind the
            # weights; pieces 4-7 on the Pool queue (Pool is otherwise unused
            # until the first collective, which fires after tile 39).
            # No DMAs on ACT: an engine's compute is FIFO-ordered behind its
            # own DMA completions.
            sel_sb = constp.tile([128, GT, PAIRS, 2, 128], mybir.dt.float8e4,
                                 tag="sel")
            PIECE = 10 * PAIRS * 2 * 128
            for k in range(8):
                qeng = nc.sync if k < 2 else nc.gpsimd
                qeng.dma_start(
                    sel_sb[:, 10 * k:10 * (k + 1), :, :, :],
                    sel_d[:, PIECE * k:PIECE * (k + 1)].rearrange(
                        "p (t q i d) -> p t q i d", t=10, q=PAIRS, i=2))
            # non-critical constants after the sel pieces
            linw_sb = constp.tile([128, L * 2, D], mybir.dt.bfloat16, tag="linw")
            nc.sync.dma_start(
                linw_sb[:], linw_d.ap().rearrange("(k p) d -> p k d", p=128))
            if not c_zero:
                c_sb = constp.tile([128, L, D], mybir.dt.float32, tag="cvec")
                nc.sync.dma_start(c_sb[:], c_d.ap().rearrange("l p d -> p l d"))
            if not b_zero:
                linb_sb = constp.tile([128, D], mybir.dt.float32, tag="linb")
                nc.sync.dma_start(linb_sb[:], linb_d.ap())

            # stash for pass-1 partial sums (pairs 0-1, A-dst tiles only)
            stash = constp.tile([128, GTA, D], mybir.dt.bfloat16, tag="stash")

            def gemm_quant(l, bs, hT_in, hi_pl, lo_pl):
                """GEMM + dinv pre-scale + fp8 hi/lo quantize for row tiles bs."""
                for b in bs:
                    tp = psB.tile([128, D], mybir.dt.float32, tag="gemm",
                                  name=f"tp_{l}_{b}")
                    for f in range(2):
                        nc.tensor.matmul(
                            tp[:], hT_in[f][:, 128 * b:128 * (b + 1)],
                            W_sb[:, 2 * l + f, :],
                            start=(f == 0), stop=(f == 1))
                    tb = workp.tile([128, D], mybir.dt.float32, tag="tb",
                                    name=f"tb_{l}_{b}")
                    nc.vector.tensor_scalar(
                        tb[:], tp[:], dinv_sb[:, b:b + 1], None,
                        mybir.AluOpType.mult)
                    hi = hi_pl[:, b // 2, b % 2, :]
                    nc.scalar.activation(hi, tb[:],
                                         mybir.ActivationFunctionType.Copy)
                    nc.vector.tensor_tensor(
                        lo_pl[:, b // 2, b % 2, :], tb[:], hi,
                        mybir.AluOpType.subtract)

            def consume_pre(l, h, agg, hbs):
                """agg loads on Pool (fires right as RS completes in its
                stream); relu(dinv*agg (+c)) on DVE."""
                nth = NTA if h == 0 else NT - NTA
                for k in range(nth):
                    b = NTA * h + k
                    asb = workp.tile([128, D], mybir.dt.bfloat16, tag="asb",
                                     name=f"asb_{l}_{b}")
                    nc.gpsimd.dma_start(asb[:],
                                        agg[h][128 * k:128 * (k + 1), :])
                    hb = hbp.tile([128, D], mybir.dt.bfloat16, tag="hb",
                                  name=f"hb_{l}_{b}")
                    if c_zero:
                        nc.vector.tensor_scalar(
                            hb[:], asb[:], dinv_sb[:, b:b + 1], 0.0,
                            mybir.AluOpType.mult, mybir.AluOpType.max)
                    else:
                        tmp = workp.tile([128, D], mybir.dt.float32,
                                         tag="tmp", name=f"tmp_{l}_{b}")
                        nc.vector.tensor_scalar(
                            tmp[:], asb[:], dinv_sb[:, b:b + 1], None,
                            mybir.AluOpType.mult)
                        nc.vector.tensor_tensor(
                            tmp[:], tmp[:], c_sb[:, l, :],
                            mybir.AluOpType.add)
                        nc.vector.tensor_scalar(
                            hb[:], tmp[:], 0.0, None, mybir.AluOpType.max)
                    hbs[b] = hb

            def consume_post(l, h, hbs, new_hT):
                """transpose h tiles into hT (PE + ACT copies)."""
                nth = NTA if h == 0 else NT - NTA
                for k in range(nth):
                    b = NTA * h + k
                    hb = hbs[b]
                    for f in range(2):
                        tps = psT.tile([128, 128], mybir.dt.float32, tag="tr",
                                       name=f"tr_{l}_{b}_{f}")
                        nc.tensor.matmul(tps[:],
                                         hb[:, 128 * f:128 * (f + 1)],
                                         ident[:], start=True, stop=True)
                        nc.scalar.activation(
                            new_hT[f][:, 128 * b:128 * (b + 1)], tps[:],
                            mybir.ActivationFunctionType.Copy)

            # software pipeline per layer l:
            #   consume-A(l-1); GEMM-A(l); pass1(l): A-dst tiles, src pairs
            #     0-1 -> stash            [runs during RS_B(l-1)]
            #   consume-B(l-1); GEMM-B(l); pass2(l): A-dst tiles, src pairs
            #     2-4 + stash merge -> partial A; RS_A(l)
            #   fullB(l): B-dst tiles, all 10 matmuls -> partial B
            #     [runs during RS_A(l)]; RS_B(l)
            P1 = (0, 1)
            P2 = (2, 3, 4)
            GRP = 4
            hT_layers = []
            prev = None      # (agg tiles, new_hT dict) pending consume
            for l in range(L):
                hi_pl = planep.tile([128, PAIRS, 2, D], mybir.dt.float8e4,
                                    tag="hi", name=f"hi_{l}")
                lo_pl = planep.tile([128, PAIRS, 2, D], mybir.dt.float8e4,
                                    tag="lo", name=f"lo_{l}")
                if prev is not None:
                    consume_post(l - 1, 0, prev[0], prev[1])
                    hT = prev[1]
                gemm_quant(l, range(NTA), hT, hi_pl, lo_pl)
                # pass 1: A-dst tiles, src pairs 0-1 -> stash (bf16);
                # layer 0 has no RS in flight, so skip the two-pass split
                for t in (range(GTA) if l > 0 else ()):
                    ps = psA.tile([128, D], mybir.dt.float32, tag="agg",
                                  name=f"ps1_{l}_{t}")
                    for p in P1:
                        nc.tensor.matmul(ps[:], sel_sb[:, t, p, :, :],
                                         hi_pl[:, p, :, :], perf_mode=DR,
                                         start=(p == P1[0]), stop=False)
                        nc.tensor.matmul(ps[:], sel_sb[:, t, p, :, :],
                                         lo_pl[:, p, :, :], perf_mode=DR,
                                         start=False, stop=(p == P1[-1]))
                    if t % 2 == 0:
                        nc.scalar.activation(
                            stash[:, t, :], ps[:],
                            mybir.ActivationFunctionType.Copy)
                    else:
                        nc.vector.tensor_copy(stash[:, t, :], ps[:])
                if prev is not None:
                    consume_post(l - 1, 1, prev[0], prev[1])
                gemm_quant(l, range(NTA, NT), hT, hi_pl, lo_pl)
                partial = [dramp.tile([rows, D], mybir.dt.bfloat16,
                                      tag=f"part{h}", name=f"partial_{l}_{h}")
                           for h, rows in ((0, ROWS_A), (1, ROWS_B))]
                agg = [dramp.tile([rows, D], mybir.dt.bfloat16,
                                  tag=f"agg{h}", name=f"agg_{l}_{h}")
                       for h, rows in ((0, PADH_A), (1, PADH_B))]
                # pass 2: A-dst tiles, src pairs 2-4 + stash merge
                # (layer 0: single-pass full sweep instead)
                pg = None
                for t in range(GTA):
                    if l == 0:
                        ps = psA.tile([128, D], mybir.dt.float32, tag="agg",
                                      name=f"ps0_{l}_{t}")
                        for p in range(PAIRS):
                            nc.tensor.matmul(ps[:], sel_sb[:, t, p, :, :],
                                             hi_pl[:, p, :, :], perf_mode=DR,
                                             start=(p == 0), stop=False)
                        for p in range(PAIRS):
                            nc.tensor.matmul(ps[:], sel_sb[:, t, p, :, :],
                                             lo_pl[:, p, :, :], perf_mode=DR,
                                             start=False,
                                             stop=(p == PAIRS - 1))
                        if t % GRP == 0:
                            pg = partp.tile([128, GRP, D], mybir.dt.bfloat16,
                                            tag="pg", name=f"pg0_{l}_{t}")
                        slot = pg[:, t % GRP, :]
                        if t % 2 == 0:
                            nc.scalar.activation(
                                slot, ps[:], mybir.ActivationFunctionType.Copy)
                        else:
                            nc.vector.tensor_copy(slot, ps[:])
                        if t % GRP == GRP - 1:
                            th = t - GRP + 1
                            nc.sync.dma_start(
                                partial[0][128 * th:128 * (th + GRP), :]
                                .rearrange("(g p) d -> p g d", p=128),
                                pg[:])
                        continue
                    ps = psA.tile([128, D], mybir.dt.float32, tag="agg",
                                  name=f"ps2_{l}_{t}")
                    merge_pe = (t % 2 == 0)
                    for p in P2:
                        nc.tensor.matmul(ps[:], sel_sb[:, t, p, :, :],
                                         hi_pl[:, p, :, :], perf_mode=DR,
                                         start=(p == P2[0]), stop=False)
                        nc.tensor.matmul(ps[:], sel_sb[:, t, p, :, :],
                                         lo_pl[:, p, :, :], perf_mode=DR,
                                         start=False,
                                         stop=(not merge_pe and p == P2[-1]))
                    if merge_pe:
                        # merge the pass-1 stash on the PE: psum += I.T @ stash
                        nc.tensor.matmul(ps[:], ident[:], stash[:, t, :],
                                         start=False, stop=True)
                    if t % GRP == 0:
                        pg = partp.tile([128, GRP, D], mybir.dt.bfloat16,
                                        tag="pg", name=f"pgA_{l}_{t}")
                    slot = pg[:, t % GRP, :]
                    if merge_pe:
                        # plain evict on ACT (gpsimd may not touch PSUM on HW)
                        nc.scalar.activation(
                            slot, ps[:], mybir.ActivationFunctionType.Copy)
                    else:
                        # merge on DVE while evicting
                        nc.vector.tensor_tensor(slot, ps[:], stash[:, t, :],
                                                mybir.AluOpType.add)
                    if t % GRP == GRP - 1:
                        th = t - GRP + 1
                        nc.sync.dma_start(
                            partial[0][128 * th:128 * (th + GRP), :]
                            .rearrange("(g p) d -> p g d", p=128),
                            pg[:])
                nc.gpsimd.collective_compute(
                    "ReduceScatter", mybir.AluOpType.add,
                    replica_groups=[list(range(NCORES))],
                    ins=[partial[0].opt()], outs=[agg[0].opt()])
                hbs = {}
                consume_pre(l, 0, agg, hbs)
                # B-dst tiles: full 10-matmul sweep (fills the RS_A window)
                for tt in range(GTB):
                    t = GTA + tt
                    ps = psA.tile([128, D], mybir.dt.float32, tag="agg",
                                  name=f"psB_{l}_{t}")
                    for p in range(PAIRS):
                        nc.tensor.matmul(ps[:], sel_sb[:, t, p, :, :],
                                         hi_pl[:, p, :, :], perf_mode=DR,
                                         start=(p == 0), stop=False)
                    for p in range(PAIRS):
                        nc.tensor.matmul(ps[:], sel_sb[:, t, p, :, :],
                                         lo_pl[:, p, :, :], perf_mode=DR,
                                         start=False, stop=(p == PAIRS - 1))
                    if tt % GRP == 0:
                        pg = partp.tile([128, GRP, D], mybir.dt.bfloat16,
                                        tag="pg", name=f"pgB_{l}_{t}")
                    slot = pg[:, tt % GRP, :]
                    nc.scalar.activation(
                        slot, ps[:], mybir.ActivationFunctionType.Copy)
                    if tt % GRP == GRP - 1:
                        th = tt - GRP + 1
                        nc.sync.dma_start(
                            partial[1][128 * th:128 * (th + GRP), :]
                            .rearrange("(g p) d -> p g d", p=128),
                            pg[:])
                nc.gpsimd.collective_compute(
                    "ReduceScatter", mybir.AluOpType.add,
                    replica_groups=[list(range(NCORES))],
                    ins=[partial[1].opt()], outs=[agg[1].opt()])
                consume_pre(l, 1, agg, hbs)
                new_hT = {}
                for f in range(2):
                    new_hT[f] = hTp.tile([128, PPC], mybir.dt.bfloat16,
                                         tag=f"hT_{l}_{f}",
                                         name=f"hT_{l}_{f}")
                hT_layers.append(new_hT)
                prev = (hbs, new_hT)
            # final consumes + JK split per half
            consume_post(L - 1, 0, prev[0], prev[1])
            jk_tiles = list(range(NTA))
            for b in jk_tiles:
                op = psB.tile([128, D], mybir.dt.float32, tag="gemm",
                              name=f"jk_{b}")
                kk = 0
                for l in range(L):
                    for f in range(2):
                        nc.tensor.matmul(
                            op[:], hT_layers[l][f][:, 128 * b:128 * (b + 1)],
                            linw_sb[:, 2 * l + f, :],
                            start=(kk == 0), stop=(kk == 5))
                        kk += 1
                ob = workp.tile([128, D], mybir.dt.float32, tag="ob",
                                name=f"ob_{b}")
                if b_zero:
                    nc.vector.tensor_copy(ob[:], op[:])
                else:
                    nc.vector.tensor_tensor(ob[:], op[:], linb_sb[:],
                                            mybir.AluOpType.add)
                nc.sync.dma_start(out_d[128 * b:128 * (b + 1), :], ob[:])
            consume_post(L - 1, 1, prev[0], prev[1])

            # ---- JK final linear, B half
            for b in range(NTA, NT):
                op = psB.tile([128, D], mybir.dt.float32, tag="gemm")
                kk = 0
                for l in range(L):
                    for f in range(2):
                        nc.tensor.matmul(
                            op[:], hT_layers[l][f][:, 128 * b:128 * (b + 1)],
                            linw_sb[:, 2 * l + f, :],
                            start=(kk == 0), stop=(kk == 5))
                        kk += 1
                ob = workp.tile([128, D], mybir.dt.float32, tag="ob")
                if b_zero:
                    nc.vector.tensor_copy(ob[:], op[:])
                else:
                    nc.vector.tensor_tensor(ob[:], op[:], linb_sb[:],
                                            mybir.AluOpType.add)
                nc.sync.dma_start(out_d[128 * b:128 * (b + 1), :], ob[:])
    nc.compile()
    return nc


# ------------------------------------------------------------------- runner
def _build_args(pre):
    return (pre["Wp"], pre["c"], pre["lin_w"], pre["lin_b"])


def _in_maps(pre):
    sel8 = pre["sel"].astype(np.float32).astype(F8)
    return [
        {"xT": np.ascontiguousarray(pre["xT"][cc].astype(F8) if False
                                     else pre["xT"][cc].astype(BF16)),
         "sel": np.ascontiguousarray(
             sel8[cc].reshape(128, GT * PAIRS * 2 * 128)),
         "dinv": np.ascontiguousarray(pre["dinv_t"][cc])}
        for cc in range(NCORES)
    ]


def _postprocess(pre, results):
    out = np.empty((N, D), np.float32)
    owner, local = pre["owner"], pre["local"]
    for cc in range(NCORES):
        m = owner == cc
        out[m] = results[cc]["out"][local[m]]
    return out


def kernel(x, edge_index, Ws, bs, bn_scale, bn_bias, bn_mean, bn_var,
           lin_w, lin_b):
    pre = _preprocess(x, edge_index, Ws, bs, bn_scale, bn_bias, bn_mean,
                      bn_var, lin_w, lin_b)
    nc = _build(*_build_args(pre))
    in_maps = _in_maps(pre)
    kw = {}
    if _TRACE:
        kw = dict(trace=True)
    res = run_bass_kernel_spmd(nc, in_maps, core_ids=list(range(NCORES)), **kw)
    kernel.last_results = res
    return _postprocess(pre, res.results)


# revision 6
# speedup vs baseline: 1.0526x; 1.0094x over previous
"""GCN (3-layer, JK-concat) Trainium2 kernel, 8-core SPMD.

Src-sharded + ReduceScatter design:
 - Nodes split into halves A/B x 8 cores (640 padded rows per (core, half);
   global table row = 5120h + 640c + j); each core owns 1280 local rows.
 - Per layer: local GEMM t' = dinv_src * (h @ W') -> fp8 hi/lo quantize
   (hi = fp8(t'), lo = fp8(t' - hi): bf16-level accuracy at fp8 DoubleRow
   matmul rate) -> per global dst tile, DoubleRow matmuls with one-hot sel
   matrices as stationary weights scatter-accumulate edge messages in PSUM
   -> bf16 partial table in DRAM -> per-half ReduceScatter(add) -> each
   core gets its aggregated rows -> relu(dinv_dst * agg + c) -> transpose
   for the next GEMM.  BN folded into W' and c.
 - Software pipeline: pass1 (src pairs 0-1 -> SBUF stash) fills the
   RS_B(l-1) window; pass2 (pairs 2-4 + stash merge via PE identity matmul
   or DVE add) feeds RS_A(l); full-sweep B-dst tiles fill the RS_A(l)
   window.  Layer 0 (no RS in flight) runs plain full sweeps.
 - Engine placement: Pool = collectives + single batched agg loads (fire
   in stream order right as each RS completes) + most of the sel upload,
   DVE = relu/quantize/merges, ACT = evicts/transpose-copies (no DMAs:
   engine compute is FIFO-ordered behind its own DMA completions), SP =
   batched partial writes + xT/weights/outputs.
 - JK: out = sum_l hT_l.T @ lin_w_l (+ lin_b), fp32, local, split per half.
"""
import sys
sys.path.insert(0, "/opt/trn_rl_repo")
import numpy as np
import ml_dtypes

import concourse.bass as bass
import concourse.bacc as bacc
import concourse.mybir as mybir
import concourse.tile as tile
from concourse.bass_utils import run_bass_kernel_spmd

N = 10000
D = 256
L = 3
BN_EPS = 1e-5
NCORES = 8
# even halves: A = dst tiles 0-39, B = 40-79 (src chunks 0-4 / 5-9)
CPH_A, CPH_B = 625, 625       # real nodes per core per half
PADH_A, PADH_B = 640, 640     # padded rows per core per half
PPC = PADH_A + PADH_B         # local rows per core (1280)
NT = PPC // 128               # local row tiles (10)
NTA = PADH_A // 128           # local A tiles (4)
ROWS_A = NCORES * PADH_A      # 4096 global A rows
ROWS_B = NCORES * PADH_B      # 6144 global B rows
GTA = ROWS_A // 128           # 32 global A dst tiles
GT = (ROWS_A + ROWS_B) // 128  # 80 global dst tiles
GTB = GT - GTA                # 48 global B dst tiles
PAIRS = NT // 2               # 5 local src chunk-pairs
BF16 = ml_dtypes.bfloat16
F8 = ml_dtypes.float8_e4m3

_TRACE = False


# ----------------------------------------------------------------- host prep
def _preprocess(x, edge_index, Ws, bs, bn_scale, bn_bias, bn_mean, bn_var,
                lin_w, lin_b):
    src = np.asarray(edge_index[0], np.int64)
    dst = np.asarray(edge_index[1], np.int64)
    loops = np.arange(N, dtype=np.int64)
    src_f = np.concatenate([src, loops])
    dst_f = np.concatenate([dst, loops])

    deg = np.bincount(dst_f, minlength=N).astype(np.float64)
    dinv = np.where(deg > 0, 1.0 / np.sqrt(deg), 0.0).astype(np.float32)

    # node n -> (owner core, local row, global table row); first 8*CPH_A
    # nodes go to half A, rest to half B
    n = np.arange(N)
    NA = NCORES * CPH_A
    in_a = n < NA
    ia = n
    ib = n - NA
    owner = np.where(in_a, ia // CPH_A, ib // CPH_B)
    local = np.where(in_a, ia % CPH_A, PADH_A + ib % CPH_B)
    grow = np.where(in_a, PADH_A * owner + ia % CPH_A,
                    ROWS_A + PADH_B * owner + ib % CPH_B)

    # per-core xT and dinv tiles
    xs = np.asarray(x, np.float32)
    xT = np.zeros((NCORES, D, PPC), np.float32)
    xT[owner, :, local] = xs
    dinv_t = np.zeros((NCORES, 128, NT), np.float32)
    dinv_t[owner, local % 128, local // 128] = dinv

    # sel blocks: [core, src_part, dst_tile, pair, half_of_pair, dst_part]
    oc = owner[src_f]
    sl = local[src_f]
    dr = grow[dst_f]
    sel = np.zeros((NCORES, 128, GT, PAIRS, 2, 128), np.uint8)
    np.add.at(sel, (oc, sl % 128, dr // 128, sl // 256, (sl // 128) % 2,
                    dr % 128), 1)
    assert sel.max() < 16

    # BN folding
    rs = 1.0 / np.sqrt(np.asarray(bn_var, np.float64) + BN_EPS)
    colscale = rs * np.asarray(bn_scale, np.float64)           # [L,D]
    Wp = np.asarray(Ws, np.float64) * colscale[:, None, :]     # [L,D,D]
    c = ((np.asarray(bs, np.float64) - np.asarray(bn_mean, np.float64))
         * colscale + np.asarray(bn_bias, np.float64))         # [L,D]

    return dict(
        sel=sel, xT=xT, dinv_t=dinv_t,
        Wp=Wp.astype(np.float32), c=c.astype(np.float32),
        lin_w=np.asarray(lin_w, np.float32), lin_b=np.asarray(lin_b, np.float32),
        owner=owner, local=local,
    )


# -------------------------------------------------------------- device build
def _build(Wp, c, lin_w, lin_b):
    c_zero = bool(np.all(np.abs(c) < 1e-12))
    b_zero = bool(np.all(np.abs(lin_b) < 1e-12))

    nc = bacc.Bacc("TRN2", target_bir_lowering=False, debug=False,
                   enable_asserts=True, num_devices=NCORES)
    xT_d = nc.dram_tensor("xT", [D, PPC], mybir.dt.bfloat16,
                          kind="ExternalInput")
    sel_d = nc.dram_tensor("sel", [128, GT * PAIRS * 2 * 128], mybir.dt.float8e4,
                           kind="ExternalInput")
    dinv_d = nc.dram_tensor("dinv", [128, NT], mybir.dt.float32,
                            kind="ExternalInput")
    out_d = nc.dram_tensor("out", [PPC, D], mybir.dt.float32,
                           kind="ExternalOutput")

    ident_d = nc.inline_tensor(np.eye(128, dtype=BF16), name="ident")
    Wp_d = nc.inline_tensor(Wp.astype(BF16), name="Wp")            # [L,D,D]
    linw_d = nc.inline_tensor(lin_w.astype(BF16), name="linw")     # [768,D]
    if not c_zero:
        c_d = nc.inline_tensor(
            np.broadcast_to(c[:, None, :], (L, 128, D)).copy(), name="cvec")
    if not b_zero:
        linb_d = nc.inline_tensor(
            np.broadcast_to(lin_b[None, :], (128, D)).copy(), name="linb")

    DR = mybir.MatmulPerfMode.DoubleRow

    with tile.TileContext(nc) as tc:
        with (
            tc.tile_pool(name="const", bufs=1) as constp,
            tc.tile_pool(name="hT", bufs=1) as hTp,
            tc.tile_pool(name="plane", bufs=2) as planep,
            tc.tile_pool(name="work", bufs=3) as workp,
            tc.tile_pool(name="part", bufs=2) as partp,
            tc.tile_pool(name="hb", bufs=10) as hbp,
            tc.tile_pool(name="dram", bufs=2, space="DRAM") as dramp,
            tc.tile_pool(name="psA", bufs=3, space="PSUM") as psA,
            tc.tile_pool(name="psB", bufs=2, space="PSUM") as psB,
            tc.tile_pool(name="psT", bufs=2, space="PSUM") as psT,
        ):
            # ---- x^T load first (it gates layer-0 GEMM), SP queue; host
            # pre-casts to bf16 so it loads straight into hT tiles
            hT = {}
            for f in range(2):
                t = hTp.tile([128, PPC], mybir.dt.bfloat16, tag=f"hT_x_{f}",
                             name=f"hT_x_{f}")
                nc.sync.dma_start(t[:], xT_d[128 * f:128 * (f + 1), :])
                hT[f] = t
            # ---- critical constants (SP queue, right after xT)
            ident = constp.tile([128, 128], mybir.dt.bfloat16, tag="ident")
            nc.sync.dma_start(ident[:], ident_d[:])
            dinv_sb = constp.tile([128, NT], mybir.dt.float32, tag="dinv")
            nc.sync.dma_start(dinv_sb[:], dinv_d[:])
            W_sb = constp.tile([128, L * 2, D], mybir.dt.bfloat16, tag="W")
            nc.sync.dma_start(
                W_sb[:], Wp_d.ap().rearrange("l (h p) d -> p (l h) d", p=128))
            # sel matrices: pieces 0-3 (dst tiles 0-39) on SP right behind the
            # weights; pieces 4-7 on the Pool queue (Pool is otherwise unused
            # until the first collective, which fires after tile 39).
            # No DMAs on ACT: an engine's compute is FIFO-ordered behind its
            # own DMA completions.
            sel_sb = constp.tile([128, GT, PAIRS, 2, 128], mybir.dt.float8e4,
                                 tag="sel")
            PIECE = 10 * PAIRS * 2 * 128
            for k in range(8):
                qeng = nc.sync if k < 2 else nc.gpsimd
                qeng.dma_start(
                    sel_sb[:, 10 * k:10 * (k + 1), :, :, :],
                    sel_d[:, PIECE * k:PIECE * (k + 1)].rearrange(
                        "p (t q i d) -> p t q i d", t=10, q=PAIRS, i=2))
            # non-critical constants after the sel pieces
            linw_sb = constp.tile([128, L * 2, D], mybir.dt.bfloat16, tag="linw")
            nc.sync.dma_start(
                linw_sb[:], linw_d.ap().rearrange("(k p) d -> p k d", p=128))
            if not c_zero:
                c_sb = constp.tile([128, L, D], mybir.dt.float32, tag="cvec")
                nc.sync.dma_start(c_sb[:], c_d.ap().rearrange("l p d -> p l d"))
            if not b_zero:
                linb_sb = constp.tile([128, D], mybir.dt.float32, tag="linb")
                nc.sync.dma_start(linb_sb[:], linb_d.ap())

            # stash for pass-1 partial sums (pairs 0-1, A-dst tiles only)
            stash = constp.tile([128, GTA, D], mybir.dt.bfloat16, tag="stash")

            def gemm_quant(l, bs, hT_in, hi_pl, lo_pl):
                """GEMM + dinv pre-scale + fp8 hi/lo quantize for row tiles bs."""
                for b in bs:
                    tp = psB.tile([128, D], mybir.dt.float32, tag="gemm",
                                  name=f"tp_{l}_{b}")
                    for f in range(2):
                        nc.tensor.matmul(
                            tp[:], hT_in[f][:, 128 * b:128 * (b + 1)],
                            W_sb[:, 2 * l + f, :],
                            start=(f == 0), stop=(f == 1))
                    tb = workp.tile([128, D], mybir.dt.float32, tag="tb",
                                    name=f"tb_{l}_{b}")
                    nc.vector.tensor_scalar(
                        tb[:], tp[:], dinv_sb[:, b:b + 1], None,
                        mybir.AluOpType.mult)
                    hi = hi_pl[:, b // 2, b % 2, :]
                    nc.scalar.activation(hi, tb[:],
                                         mybir.ActivationFunctionType.Copy)
                    nc.vector.tensor_tensor(
                        lo_pl[:, b // 2, b % 2, :], tb[:], hi,
                        mybir.AluOpType.subtract)

            def consume_pre(l, h, agg, hbs):
                """one batched agg load on Pool (fires right as RS completes
                in its stream); relu(dinv*agg (+c)) on DVE."""
                nth = NTA if h == 0 else NT - NTA
                asb = workp.tile([128, nth, D], mybir.dt.bfloat16, tag="asb",
                                 name=f"asb_{l}_{h}")
                nc.gpsimd.dma_start(
                    asb[:],
                    agg[h][:].rearrange("(k p) d -> p k d", p=128))
                for k in range(nth):
                    b = NTA * h + k
                    hb = hbp.tile([128, D], mybir.dt.bfloat16, tag="hb",
                                  name=f"hb_{l}_{b}")
                    if c_zero:
                        nc.vector.tensor_scalar(
                            hb[:], asb[:, k, :], dinv_sb[:, b:b + 1], 0.0,
                            mybir.AluOpType.mult, mybir.AluOpType.max)
                    else:
                        tmp = workp.tile([128, D], mybir.dt.float32,
                                         tag="tmp", name=f"tmp_{l}_{b}")
                        nc.vector.tensor_scalar(
                            tmp[:], asb[:, k, :], dinv_sb[:, b:b + 1], None,
                            mybir.AluOpType.mult)
                        nc.vector.tensor_tensor(
                            tmp[:], tmp[:], c_sb[:, l, :],
                            mybir.AluOpType.add)
                        nc.vector.tensor_scalar(
                            hb[:], tmp[:], 0.0, None, mybir.AluOpType.max)
                    hbs[b] = hb

            def consume_post(l, h, hbs, new_hT):
                """transpose h tiles into hT (PE + ACT copies)."""
                nth = NTA if h == 0 else NT - NTA
                for k in range(nth):
                    b = NTA * h + k
                    hb = hbs[b]
                    for f in range(2):
                        tps = psT.tile([128, 128], mybir.dt.float32, tag="tr",
                                       name=f"tr_{l}_{b}_{f}")
                        nc.tensor.matmul(tps[:],
                                         hb[:, 128 * f:128 * (f + 1)],
                                         ident[:], start=True, stop=True)
                        nc.scalar.activation(
                            new_hT[f][:, 128 * b:128 * (b + 1)], tps[:],
                            mybir.ActivationFunctionType.Copy)

            # software pipeline per layer l:
            #   consume-A(l-1); GEMM-A(l); pass1(l): A-dst tiles, src pairs
            #     0-1 -> stash            [runs during RS_B(l-1)]
            #   consume-B(l-1); GEMM-B(l); pass2(l): A-dst tiles, src pairs
            #     2-4 + stash merge -> partial A; RS_A(l)
            #   fullB(l): B-dst tiles, all 10 matmuls -> partial B
            #     [runs during RS_A(l)]; RS_B(l)
            P1 = (0, 1)
            P2 = (2, 3, 4)
            GRP = 4
            hT_layers = []
            prev = None      # (agg tiles, new_hT dict) pending consume
            for l in range(L):
                hi_pl = planep.tile([128, PAIRS, 2, D], mybir.dt.float8e4,
                                    tag="hi", name=f"hi_{l}")
                lo_pl = planep.tile([128, PAIRS, 2, D], mybir.dt.float8e4,
                                    tag="lo", name=f"lo_{l}")
                if prev is not None:
                    consume_post(l - 1, 0, prev[0], prev[1])
                    hT = prev[1]
                gemm_quant(l, range(NTA), hT, hi_pl, lo_pl)
                # pass 1: A-dst tiles, src pairs 0-1 -> stash (bf16);
                # layer 0 has no RS in flight, so skip the two-pass split
                for t in (range(GTA) if l > 0 else ()):
                    ps = psA.tile([128, D], mybir.dt.float32, tag="agg",
                                  name=f"ps1_{l}_{t}")
                    for p in P1:
                        nc.tensor.matmul(ps[:], sel_sb[:, t, p, :, :],
                                         hi_pl[:, p, :, :], perf_mode=DR,
                                         start=(p == P1[0]), stop=False)
                        nc.tensor.matmul(ps[:], sel_sb[:, t, p, :, :],
                                         lo_pl[:, p, :, :], perf_mode=DR,
                                         start=False, stop=(p == P1[-1]))
                    if t % 2 == 0:
                        nc.scalar.activation(
                            stash[:, t, :], ps[:],
                            mybir.ActivationFunctionType.Copy)
                    else:
                        nc.vector.tensor_copy(stash[:, t, :], ps[:])
                if prev is not None:
                    consume_post(l - 1, 1, prev[0], prev[1])
                gemm_quant(l, range(NTA, NT), hT, hi_pl, lo_pl)
                partial = [dramp.tile([rows, D], mybir.dt.bfloat16,
                                      tag=f"part{h}", name=f"partial_{l}_{h}")
                           for h, rows in ((0, ROWS_A), (1, ROWS_B))]
                agg = [dramp.tile([rows, D], mybir.dt.bfloat16,
                                  tag=f"agg{h}", name=f"agg_{l}_{h}")
                       for h, rows in ((0, PADH_A), (1, PADH_B))]
                # pass 2: A-dst tiles, src pairs 2-4 + stash merge
                # (layer 0: single-pass full sweep instead)
                pg = None
                for t in range(GTA):
                    if l == 0:
                        ps = psA.tile([128, D], mybir.dt.float32, tag="agg",
                                      name=f"ps0_{l}_{t}")
                        for p in range(PAIRS):
                            nc.tensor.matmul(ps[:], sel_sb[:, t, p, :, :],
                                             hi_pl[:, p, :, :], perf_mode=DR,
                                             start=(p == 0), stop=False)
                        for p in range(PAIRS):
                            nc.tensor.matmul(ps[:], sel_sb[:, t, p, :, :],
                                             lo_pl[:, p, :, :], perf_mode=DR,
                                             start=False,
                                             stop=(p == PAIRS - 1))
                        if t % GRP == 0:
                            pg = partp.tile([128, GRP, D], mybir.dt.bfloat16,
                                            tag="pg", name=f"pg0_{l}_{t}")
                        slot = pg[:, t % GRP, :]
                        if t % 2 == 0:
                            nc.scalar.activation(
                                slot, ps[:], mybir.ActivationFunctionType.Copy)
                        else:
                            nc.vector.tensor_copy(slot, ps[:])
                        if t % GRP == GRP - 1:
                            th = t - GRP + 1
                            nc.sync.dma_start(
                                partial[0][128 * th:128 * (th + GRP), :]
                                .rearrange("(g p) d -> p g d", p=128),
                                pg[:])
                        continue
                    ps = psA.tile([128, D], mybir.dt.float32, tag="agg",
                                  name=f"ps2_{l}_{t}")
                    merge_pe = (t % 3 == 0)
                    for p in P2:
                        nc.tensor.matmul(ps[:], sel_sb[:, t, p, :, :],
                                         hi_pl[:, p, :, :], perf_mode=DR,
                                         start=(p == P2[0]), stop=False)
                        nc.tensor.matmul(ps[:], sel_sb[:, t, p, :, :],
                                         lo_pl[:, p, :, :], perf_mode=DR,
                                         start=False,
                                         stop=(not merge_pe and p == P2[-1]))
                    if merge_pe:
                        # merge the pass-1 stash on the PE: psum += I.T @ stash
                        nc.tensor.matmul(ps[:], ident[:], stash[:, t, :],
                                         start=False, stop=True)
                    if t % GRP == 0:
                        pg = partp.tile([128, GRP, D], mybir.dt.bfloat16,
                                        tag="pg", name=f"pgA_{l}_{t}")
                    slot = pg[:, t % GRP, :]
                    if merge_pe:
                        # plain evict on ACT (gpsimd may not touch PSUM on HW)
                        nc.scalar.activation(
                            slot, ps[:], mybir.ActivationFunctionType.Copy)
                    else:
                        # merge on DVE while evicting
                        nc.vector.tensor_tensor(slot, ps[:], stash[:, t, :],
                                                mybir.AluOpType.add)
                    if t % GRP == GRP - 1:
                        th = t - GRP + 1
                        nc.sync.dma_start(
                            partial[0][128 * th:128 * (th + GRP), :]
                            .rearrange("(g p) d -> p g d", p=128),
                            pg[:])
                nc.gpsimd.collective_compute(
                    "ReduceScatter", mybir.AluOpType.add,
                    replica_groups=[list(range(NCORES))],
                    ins=[partial[0].opt()], outs=[agg[0].opt()])
                hbs = {}
                consume_pre(l, 0, agg, hbs)
                # B-dst tiles: full 10-matmul sweep (fills the RS_A window)
                for tt in range(GTB):
                    t = GTA + tt
                    ps = psA.tile([128, D], mybir.dt.float32, tag="agg",
                                  name=f"psB_{l}_{t}")
                    for p in range(PAIRS):
                        nc.tensor.matmul(ps[:], sel_sb[:, t, p, :, :],
                                         hi_pl[:, p, :, :], perf_mode=DR,
                                         start=(p == 0), stop=False)
                    for p in range(PAIRS):
                        nc.tensor.matmul(ps[:], sel_sb[:, t, p, :, :],
                                         lo_pl[:, p, :, :], perf_mode=DR,
                                         start=False, stop=(p == PAIRS - 1))
                    if tt % GRP == 0:
                        pg = partp.tile([128, GRP, D], mybir.dt.bfloat16,
                                        tag="pg", name=f"pgB_{l}_{t}")
                    slot = pg[:, tt % GRP, :]
                    nc.scalar.activation(
                        slot, ps[:], mybir.ActivationFunctionType.Copy)
                    if tt % GRP == GRP - 1:
                        th = tt - GRP + 1
                        nc.sync.dma_start(
                            partial[1][128 * th:128 * (th + GRP), :]
                            .rearrange("(g p) d -> p g d", p=128),
                            pg[:])
                nc.gpsimd.collective_compute(
                    "ReduceScatter", mybir.AluOpType.add,
                    replica_groups=[list(range(NCORES))],
                    ins=[partial[1].opt()], outs=[agg[1].opt()])
                consume_pre(l, 1, agg, hbs)
                new_hT = {}
                for f in range(2):
                    new_hT[f] = hTp.tile([128, PPC], mybir.dt.bfloat16,
                                         tag=f"hT_{l}_{f}",
                                         name=f"hT_{l}_{f}")
                hT_layers.append(new_hT)
                prev = (hbs, new_hT)
            # final consumes + JK split per half
            consume_post(L - 1, 0, prev[0], prev[1])
            jk_tiles = list(range(NTA))
            for b in jk_tiles:
                op = psB.tile([128, D], mybir.dt.float32, tag="gemm",
                              name=f"jk_{b}")
                kk = 0
                for l in range(L):
                    for f in range(2):
                        nc.tensor.matmul(
                            op[:], hT_layers[l][f][:, 128 * b:128 * (b + 1)],
                            linw_sb[:, 2 * l + f, :],
                            start=(kk == 0), stop=(kk == 5))
                        kk += 1
                ob = workp.tile([128, D], mybir.dt.float32, tag="ob",
                                name=f"ob_{b}")
                if b_zero:
                    nc.vector.tensor_copy(ob[:], op[:])
                else:
                    nc.vector.tensor_tensor(ob[:], op[:], linb_sb[:],
                                            mybir.AluOpType.add)
                nc.sync.dma_start(out_d[128 * b:128 * (b + 1), :], ob[:])
            consume_post(L - 1, 1, prev[0], prev[1])

            # ---- JK final linear, B half
            for b in range(NTA, NT):
                op = psB.tile([128, D], mybir.dt.float32, tag="gemm")
                kk = 0
                for l in range(L):
                    for f in range(2):
                        nc.tensor.matmul(
                            op[:], hT_layers[l][f][:, 128 * b:128 * (b + 1)],
                            linw_sb[:, 2 * l + f, :],
                            start=(kk == 0), stop=(kk == 5))
                        kk += 1
                ob = workp.tile([128, D], mybir.dt.float32, tag="ob")
                if b_zero:
                    nc.vector.tensor_copy(ob[:], op[:])
                else:
                    nc.vector.tensor_tensor(ob[:], op[:], linb_sb[:],
                                            mybir.AluOpType.add)
                nc.sync.dma_start(out_d[128 * b:128 * (b + 1), :], ob[:])
    nc.compile()
    return nc


# ------------------------------------------------------------------- runner
def _build_args(pre):
    return (pre["Wp"], pre["c"], pre["lin_w"], pre["lin_b"])


def _in_maps(pre):
    sel8 = pre["sel"].astype(np.float32).astype(F8)
    return [
        {"xT": np.ascontiguousarray(pre["xT"][cc].astype(F8) if False
                                     else pre["xT"][cc].astype(BF16)),
         "sel": np.ascontiguousarray(
             sel8[cc].reshape(128, GT * PAIRS * 2 * 128)),
         "dinv": np.ascontiguousarray(pre["dinv_t"][cc])}
        for cc in range(NCORES)
    ]


def _postprocess(pre, results):
    out = np.empty((N, D), np.float32)
    owner, local = pre["owner"], pre["local"]
    for cc in range(NCORES):
        m = owner == cc
        out[m] = results[cc]["out"][local[m]]
    return out


def kernel(x, edge_index, Ws, bs, bn_scale, bn_bias, bn_mean, bn_var,
           lin_w, lin_b):
    pre = _preprocess(x, edge_index, Ws, bs, bn_scale, bn_bias, bn_mean,
                      bn_var, lin_w, lin_b)
    nc = _build(*_build_args(pre))
    in_maps = _in_maps(pre)
    kw = {}
    if _TRACE:
        kw = dict(trace=True)
    res = run_bass_kernel_spmd(nc, in_maps, core_ids=list(range(NCORES)), **kw)
    kernel.last_results = res
    return _postprocess(pre, res.results)


# revision 7
# speedup vs baseline: 1.0707x; 1.0172x over previous
"""GCN (3-layer, JK-concat) Trainium2 kernel, 8-core SPMD.

Src-sharded + ReduceScatter design:
 - Nodes split into halves A/B x 8 cores (640 padded rows per (core, half);
   global table row = 5120h + 640c + j); each core owns 1280 local rows.
 - Per layer: local GEMM t' = dinv_src * (h @ W') -> fp8 hi/lo quantize
   (hi = fp8(t'), lo = fp8(t' - hi): bf16-level accuracy at fp8 DoubleRow
   matmul rate) -> per global dst tile, DoubleRow matmuls with one-hot sel
   matrices as stationary weights scatter-accumulate edge messages in PSUM
   -> bf16 partial table in DRAM -> per-half ReduceScatter(add) -> each
   core gets its aggregated rows -> relu(dinv_dst * agg + c) -> transpose
   for the next GEMM.  BN folded into W' and c.
 - Software pipeline: pass1 (src pairs 0-1 -> SBUF stash) fills the
   RS_B(l-1) window; pass2 (pairs 2-4 + stash merge via PE identity matmul
   or DVE add) feeds RS_A(l); full-sweep B-dst tiles fill the RS_A(l)
   window.  Layer 0 (no RS in flight) runs plain full sweeps.
 - Engine placement: Pool = collectives + single batched agg loads (fire
   in stream order right as each RS completes) + most of the sel upload,
   DVE = relu/quantize/merges, ACT = evicts/transpose-copies (no DMAs:
   engine compute is FIFO-ordered behind its own DMA completions), SP =
   batched partial writes + xT/weights/outputs.
 - JK: out = sum_l hT_l.T @ lin_w_l (+ lin_b), fp32, local, split per half.
"""
import sys
sys.path.insert(0, "/opt/trn_rl_repo")
import numpy as np
import ml_dtypes

import concourse.bass as bass
import concourse.bacc as bacc
import concourse.mybir as mybir
import concourse.tile as tile
from concourse.bass_utils import run_bass_kernel_spmd

N = 10000
D = 256
L = 3
BN_EPS = 1e-5
NCORES = 8
# even halves: A = dst tiles 0-39, B = 40-79 (src chunks 0-4 / 5-9)
CPH_A, CPH_B = 625, 625       # real nodes per core per half
PADH_A, PADH_B = 640, 640     # padded rows per core per half
PPC = PADH_A + PADH_B         # local rows per core (1280)
NT = PPC // 128               # local row tiles (10)
NTA = PADH_A // 128           # local A tiles (4)
ROWS_A = NCORES * PADH_A      # 4096 global A rows
ROWS_B = NCORES * PADH_B      # 6144 global B rows
GTA = ROWS_A // 128           # 32 global A dst tiles
GT = (ROWS_A + ROWS_B) // 128  # 80 global dst tiles
GTB = GT - GTA                # 48 global B dst tiles
PAIRS = NT // 2               # 5 local src chunk-pairs
BF16 = ml_dtypes.bfloat16
F8 = ml_dtypes.float8_e4m3

_TRACE = False


# ----------------------------------------------------------------- host prep
def _preprocess(x, edge_index, Ws, bs, bn_scale, bn_bias, bn_mean, bn_var,
                lin_w, lin_b):
    src = np.asarray(edge_index[0], np.int64)
    dst = np.asarray(edge_index[1], np.int64)
    loops = np.arange(N, dtype=np.int64)
    src_f = np.concatenate([src, loops])
    dst_f = np.concatenate([dst, loops])

    deg = np.bincount(dst_f, minlength=N).astype(np.float64)
    dinv = np.where(deg > 0, 1.0 / np.sqrt(deg), 0.0).astype(np.float32)

    # node n -> (owner core, local row, global table row); first 8*CPH_A
    # nodes go to half A, rest to half B
    n = np.arange(N)
    NA = NCORES * CPH_A
    in_a = n < NA
    ia = n
    ib = n - NA
    owner = np.where(in_a, ia // CPH_A, ib // CPH_B)
    local = np.where(in_a, ia % CPH_A, PADH_A + ib % CPH_B)
    grow = np.where(in_a, PADH_A * owner + ia % CPH_A,
                    ROWS_A + PADH_B * owner + ib % CPH_B)

    # per-core xT and dinv tiles
    xs = np.asarray(x, np.float32)
    xT = np.zeros((NCORES, D, PPC), np.float32)
    xT[owner, :, local] = xs
    dinv_t = np.zeros((NCORES, 128, NT), np.float32)
    dinv_t[owner, local % 128, local // 128] = dinv

    # sel blocks: [core, src_part, dst_tile, pair, half_of_pair, dst_part]
    oc = owner[src_f]
    sl = local[src_f]
    dr = grow[dst_f]
    sel = np.zeros((NCORES, 128, GT, PAIRS, 2, 128), np.uint8)
    np.add.at(sel, (oc, sl % 128, dr // 128, sl // 256, (sl // 128) % 2,
                    dr % 128), 1)
    assert sel.max() < 16

    # BN folding
    rs = 1.0 / np.sqrt(np.asarray(bn_var, np.float64) + BN_EPS)
    colscale = rs * np.asarray(bn_scale, np.float64)           # [L,D]
    Wp = np.asarray(Ws, np.float64) * colscale[:, None, :]     # [L,D,D]
    c = ((np.asarray(bs, np.float64) - np.asarray(bn_mean, np.float64))
         * colscale + np.asarray(bn_bias, np.float64))         # [L,D]

    return dict(
        sel=sel, xT=xT, dinv_t=dinv_t,
        Wp=Wp.astype(np.float32), c=c.astype(np.float32),
        lin_w=np.asarray(lin_w, np.float32), lin_b=np.asarray(lin_b, np.float32),
        owner=owner, local=local,
    )


# -------------------------------------------------------------- device build
def _build(Wp, c, lin_w, lin_b):
    c_zero = bool(np.all(np.abs(c) < 1e-12))
    b_zero = bool(np.all(np.abs(lin_b) < 1e-12))

    nc = bacc.Bacc("TRN2", target_bir_lowering=False, debug=False,
                   enable_asserts=True, num_devices=NCORES)
    xT_d = nc.dram_tensor("xT", [D, PPC], mybir.dt.bfloat16,
                          kind="ExternalInput")
    sel_d = nc.dram_tensor("sel", [128, GT * PAIRS * 2 * 128], mybir.dt.float8e4,
                           kind="ExternalInput")
    dinv_d = nc.dram_tensor("dinv", [128, NT], mybir.dt.float32,
                            kind="ExternalInput")
    out_d = nc.dram_tensor("out", [PPC, D], mybir.dt.float32,
                           kind="ExternalOutput")

    ident_d = nc.inline_tensor(np.eye(128, dtype=BF16), name="ident")
    Wp_d = nc.inline_tensor(Wp.astype(BF16), name="Wp")            # [L,D,D]
    linw_d = nc.inline_tensor(lin_w.astype(BF16), name="linw")     # [768,D]
    if not c_zero:
        c_d = nc.inline_tensor(
            np.broadcast_to(c[:, None, :], (L, 128, D)).copy(), name="cvec")
    if not b_zero:
        linb_d = nc.inline_tensor(
            np.broadcast_to(lin_b[None, :], (128, D)).copy(), name="linb")

    DR = mybir.MatmulPerfMode.DoubleRow

    with tile.TileContext(nc) as tc:
        with (
            tc.tile_pool(name="const", bufs=1) as constp,
            tc.tile_pool(name="hT", bufs=1) as hTp,
            tc.tile_pool(name="plane", bufs=2) as planep,
            tc.tile_pool(name="work", bufs=3) as workp,
            tc.tile_pool(name="part", bufs=2) as partp,
            tc.tile_pool(name="hb", bufs=10) as hbp,
            tc.tile_pool(name="dram", bufs=2, space="DRAM") as dramp,
            tc.tile_pool(name="psA", bufs=4, space="PSUM") as psA,
            tc.tile_pool(name="psB", bufs=2, space="PSUM") as psB,
            tc.tile_pool(name="psT", bufs=2, space="PSUM") as psT,
        ):
            # ---- x^T load first (it gates layer-0 GEMM), SP queue; host
            # pre-casts to bf16 so it loads straight into hT tiles
            hT = {}
            for f in range(2):
                t = hTp.tile([128, PPC], mybir.dt.bfloat16, tag=f"hT_x_{f}",
                             name=f"hT_x_{f}")
                nc.sync.dma_start(t[:], xT_d[128 * f:128 * (f + 1), :])
                hT[f] = t
            # ---- critical constants (SP queue, right after xT)
            ident = constp.tile([128, 128], mybir.dt.bfloat16, tag="ident")
            nc.sync.dma_start(ident[:], ident_d[:])
            dinv_sb = constp.tile([128, NT], mybir.dt.float32, tag="dinv")
            nc.sync.dma_start(dinv_sb[:], dinv_d[:])
            W_sb = constp.tile([128, L * 2, D], mybir.dt.bfloat16, tag="W")
            nc.sync.dma_start(
                W_sb[:], Wp_d.ap().rearrange("l (h p) d -> p (l h) d", p=128))
            # sel matrices: pieces 0-3 (dst tiles 0-39) on SP right behind the
            # weights; pieces 4-7 on the Pool queue (Pool is otherwise unused
            # until the first collective, which fires after tile 39).
            # No DMAs on ACT: an engine's compute is FIFO-ordered behind its
            # own DMA completions.
            sel_sb = constp.tile([128, GT, PAIRS, 2, 128], mybir.dt.float8e4,
                                 tag="sel")
            PIECE = 10 * PAIRS * 2 * 128
            for k in range(8):
                qeng = nc.sync if k < 2 else nc.gpsimd
                qeng.dma_start(
                    sel_sb[:, 10 * k:10 * (k + 1), :, :, :],
                    sel_d[:, PIECE * k:PIECE * (k + 1)].rearrange(
                        "p (t q i d) -> p t q i d", t=10, q=PAIRS, i=2))
            # non-critical constants after the sel pieces
            linw_sb = constp.tile([128, L * 2, D], mybir.dt.bfloat16, tag="linw")
            nc.sync.dma_start(
                linw_sb[:], linw_d.ap().rearrange("(k p) d -> p k d", p=128))
            if not c_zero:
                c_sb = constp.tile([128, L, D], mybir.dt.float32, tag="cvec")
                nc.sync.dma_start(c_sb[:], c_d.ap().rearrange("l p d -> p l d"))
            if not b_zero:
                linb_sb = constp.tile([128, D], mybir.dt.float32, tag="linb")
                nc.sync.dma_start(linb_sb[:], linb_d.ap())

            # stash for pass-1 partial sums (pairs 0-1, A-dst tiles only)
            stash = constp.tile([128, GTA, D], mybir.dt.bfloat16, tag="stash")

            def gemm_quant(l, bs, hT_in, hi_pl, lo_pl):
                """GEMM + dinv pre-scale + fp8 hi/lo quantize for row tiles bs."""
                for b in bs:
                    tp = psB.tile([128, D], mybir.dt.float32, tag="gemm",
                                  name=f"tp_{l}_{b}")
                    for f in range(2):
                        nc.tensor.matmul(
                            tp[:], hT_in[f][:, 128 * b:128 * (b + 1)],
                            W_sb[:, 2 * l + f, :],
                            start=(f == 0), stop=(f == 1))
                    tb = workp.tile([128, D], mybir.dt.float32, tag="tb",
                                    name=f"tb_{l}_{b}")
                    nc.vector.tensor_scalar(
                        tb[:], tp[:], dinv_sb[:, b:b + 1], None,
                        mybir.AluOpType.mult)
                    hi = hi_pl[:, b // 2, b % 2, :]
                    nc.scalar.activation(hi, tb[:],
                                         mybir.ActivationFunctionType.Copy)
                    nc.vector.tensor_tensor(
                        lo_pl[:, b // 2, b % 2, :], tb[:], hi,
                        mybir.AluOpType.subtract)

            def consume_pre(l, h, agg, hbs):
                """one batched agg load on Pool (fires right as RS completes
                in its stream); relu(dinv*agg (+c)) on DVE."""
                nth = NTA if h == 0 else NT - NTA
                asb = workp.tile([128, nth, D], mybir.dt.bfloat16, tag="asb",
                                 name=f"asb_{l}_{h}")
                nc.gpsimd.dma_start(
                    asb[:],
                    agg[h][:].rearrange("(k p) d -> p k d", p=128))
                for k in range(nth):
                    b = NTA * h + k
                    hb = hbp.tile([128, D], mybir.dt.bfloat16, tag="hb",
                                  name=f"hb_{l}_{b}")
                    if c_zero:
                        nc.vector.tensor_scalar(
                            hb[:], asb[:, k, :], dinv_sb[:, b:b + 1], 0.0,
                            mybir.AluOpType.mult, mybir.AluOpType.max)
                    else:
                        tmp = workp.tile([128, D], mybir.dt.float32,
                                         tag="tmp", name=f"tmp_{l}_{b}")
                        nc.vector.tensor_scalar(
                            tmp[:], asb[:, k, :], dinv_sb[:, b:b + 1], None,
                            mybir.AluOpType.mult)
                        nc.vector.tensor_tensor(
                            tmp[:], tmp[:], c_sb[:, l, :],
                            mybir.AluOpType.add)
                        nc.vector.tensor_scalar(
                            hb[:], tmp[:], 0.0, None, mybir.AluOpType.max)
                    hbs[b] = hb

            def consume_post(l, h, hbs, new_hT):
                """transpose h tiles into hT (PE + ACT copies)."""
                nth = NTA if h == 0 else NT - NTA
                for k in range(nth):
                    b = NTA * h + k
                    hb = hbs[b]
                    for f in range(2):
                        tps = psT.tile([128, 128], mybir.dt.float32, tag="tr",
                                       name=f"tr_{l}_{b}_{f}")
                        nc.tensor.matmul(tps[:],
                                         hb[:, 128 * f:128 * (f + 1)],
                                         ident[:], start=True, stop=True)
                        nc.scalar.activation(
                            new_hT[f][:, 128 * b:128 * (b + 1)], tps[:],
                            mybir.ActivationFunctionType.Copy)

            # software pipeline per layer l:
            #   consume-A(l-1); GEMM-A(l); pass1(l): A-dst tiles, src pairs
            #     0-1 -> stash            [runs during RS_B(l-1)]
            #   consume-B(l-1); GEMM-B(l); pass2(l): A-dst tiles, src pairs
            #     2-4 + stash merge -> partial A; RS_A(l)
            #   fullB(l): B-dst tiles, all 10 matmuls -> partial B
            #     [runs during RS_A(l)]; RS_B(l)
            P1 = (0, 1)
            P2 = (2, 3, 4)
            GRP = 4
            hT_layers = []
            prev = None      # (agg tiles, new_hT dict) pending consume
            for l in range(L):
                hi_pl = planep.tile([128, PAIRS, 2, D], mybir.dt.float8e4,
                                    tag="hi", name=f"hi_{l}")
                lo_pl = planep.tile([128, PAIRS, 2, D], mybir.dt.float8e4,
                                    tag="lo", name=f"lo_{l}")
                if prev is not None:
                    consume_post(l - 1, 0, prev[0], prev[1])
                    hT = prev[1]
                gemm_quant(l, range(NTA), hT, hi_pl, lo_pl)
                # pass 1: A-dst tiles, src pairs 0-1 -> stash (bf16);
                # layer 0 has no RS in flight, so skip the two-pass split
                for t in (range(GTA) if l > 0 else ()):
                    ps = psA.tile([128, D], mybir.dt.float32, tag="agg",
                                  name=f"ps1_{l}_{t}")
                    for p in P1:
                        nc.tensor.matmul(ps[:], sel_sb[:, t, p, :, :],
                                         hi_pl[:, p, :, :], perf_mode=DR,
                                         start=(p == P1[0]), stop=False)
                        nc.tensor.matmul(ps[:], sel_sb[:, t, p, :, :],
                                         lo_pl[:, p, :, :], perf_mode=DR,
                                         start=False, stop=(p == P1[-1]))
                    if t % 2 == 0:
                        nc.scalar.activation(
                            stash[:, t, :], ps[:],
                            mybir.ActivationFunctionType.Copy)
                    else:
                        nc.vector.tensor_copy(stash[:, t, :], ps[:])
                if prev is not None:
                    consume_post(l - 1, 1, prev[0], prev[1])
                gemm_quant(l, range(NTA, NT), hT, hi_pl, lo_pl)
                partial = [dramp.tile([rows, D], mybir.dt.bfloat16,
                                      tag=f"part{h}", name=f"partial_{l}_{h}")
                           for h, rows in ((0, ROWS_A), (1, ROWS_B))]
                agg = [dramp.tile([rows, D], mybir.dt.bfloat16,
                                  tag=f"agg{h}", name=f"agg_{l}_{h}")
                       for h, rows in ((0, PADH_A), (1, PADH_B))]
                # pass 2: A-dst tiles, src pairs 2-4 + stash merge
                # (layer 0: single-pass full sweep instead)
                pg = None
                for t in range(GTA):
                    if l == 0:
                        ps = psA.tile([128, D], mybir.dt.float32, tag="agg",
                                      name=f"ps0_{l}_{t}")
                        for p in range(PAIRS):
                            nc.tensor.matmul(ps[:], sel_sb[:, t, p, :, :],
                                             hi_pl[:, p, :, :], perf_mode=DR,
                                             start=(p == 0), stop=False)
                        for p in range(PAIRS):
                            nc.tensor.matmul(ps[:], sel_sb[:, t, p, :, :],
                                             lo_pl[:, p, :, :], perf_mode=DR,
                                             start=False,
                                             stop=(p == PAIRS - 1))
                        if t % GRP == 0:
                            pg = partp.tile([128, GRP, D], mybir.dt.bfloat16,
                                            tag="pg", name=f"pg0_{l}_{t}")
                        slot = pg[:, t % GRP, :]
                        if t % 2 == 0:
                            nc.scalar.activation(
                                slot, ps[:], mybir.ActivationFunctionType.Copy)
                        else:
                            nc.vector.tensor_copy(slot, ps[:])
                        if t % GRP == GRP - 1:
                            th = t - GRP + 1
                            nc.sync.dma_start(
                                partial[0][128 * th:128 * (th + GRP), :]
                                .rearrange("(g p) d -> p g d", p=128),
                                pg[:])
                        continue
                    ps = psA.tile([128, D], mybir.dt.float32, tag="agg",
                                  name=f"ps2_{l}_{t}")
                    merge_pe = (t % 3 == 0)
                    for p in P2:
                        nc.tensor.matmul(ps[:], sel_sb[:, t, p, :, :],
                                         hi_pl[:, p, :, :], perf_mode=DR,
                                         start=(p == P2[0]), stop=False)
                        nc.tensor.matmul(ps[:], sel_sb[:, t, p, :, :],
                                         lo_pl[:, p, :, :], perf_mode=DR,
                                         start=False,
                                         stop=(not merge_pe and p == P2[-1]))
                    if merge_pe:
                        # merge the pass-1 stash on the PE: psum += I.T @ stash
                        nc.tensor.matmul(ps[:], ident[:], stash[:, t, :],
                                         start=False, stop=True)
                    if t % GRP == 0:
                        pg = partp.tile([128, GRP, D], mybir.dt.bfloat16,
                                        tag="pg", name=f"pgA_{l}_{t}")
                    slot = pg[:, t % GRP, :]
                    if merge_pe:
                        # plain evict on ACT (gpsimd may not touch PSUM on HW)
                        nc.scalar.activation(
                            slot, ps[:], mybir.ActivationFunctionType.Copy)
                    else:
                        # merge on DVE while evicting
                        nc.vector.tensor_tensor(slot, ps[:], stash[:, t, :],
                                                mybir.AluOpType.add)
                    if t % GRP == GRP - 1:
                        th = t - GRP + 1
                        nc.sync.dma_start(
                            partial[0][128 * th:128 * (th + GRP), :]
                            .rearrange("(g p) d -> p g d", p=128),
                            pg[:])
                nc.gpsimd.collective_compute(
                    "ReduceScatter", mybir.AluOpType.add,
                    replica_groups=[list(range(NCORES))],
                    ins=[partial[0].opt()], outs=[agg[0].opt()])
                hbs = {}
                consume_pre(l, 0, agg, hbs)
                # B-dst tiles: full 10-matmul sweep (fills the RS_A window)
                for tt in range(GTB):
                    t = GTA + tt
                    ps = psA.tile([128, D], mybir.dt.float32, tag="agg",
                                  name=f"psB_{l}_{t}")
                    for p in range(PAIRS):
                        nc.tensor.matmul(ps[:], sel_sb[:, t, p, :, :],
                                         hi_pl[:, p, :, :], perf_mode=DR,
                                         start=(p == 0), stop=False)
                    for p in range(PAIRS):
                        nc.tensor.matmul(ps[:], sel_sb[:, t, p, :, :],
                                         lo_pl[:, p, :, :], perf_mode=DR,
                                         start=False, stop=(p == PAIRS - 1))
                    if tt % GRP == 0:
                        pg = partp.tile([128, GRP, D], mybir.dt.bfloat16,
                                        tag="pg", name=f"pgB_{l}_{t}")
                    slot = pg[:, tt % GRP, :]
                    nc.scalar.activation(
                        slot, ps[:], mybir.ActivationFunctionType.Copy)
                    if tt % GRP == GRP - 1:
                        th = tt - GRP + 1
                        nc.sync.dma_start(
                            partial[1][128 * th:128 * (th + GRP), :]
                            .rearrange("(g p) d -> p g d", p=128),
                            pg[:])
                nc.gpsimd.collective_compute(
                    "ReduceScatter", mybir.AluOpType.add,
                    replica_groups=[list(range(NCORES))],
                    ins=[partial[1].opt()], outs=[agg[1].opt()])
                consume_pre(l, 1, agg, hbs)
                new_hT = {}
                for f in range(2):
                    new_hT[f] = hTp.tile([128, PPC], mybir.dt.bfloat16,
                                         tag=f"hT_{l}_{f}",
                                         name=f"hT_{l}_{f}")
                hT_layers.append(new_hT)
                prev = (hbs, new_hT)
            # final consumes + JK split per half
            consume_post(L - 1, 0, prev[0], prev[1])
            jk_tiles = list(range(NTA))
            for b in jk_tiles:
                op = psB.tile([128, D], mybir.dt.float32, tag="gemm",
                              name=f"jk_{b}")
                kk = 0
                for l in range(L):
                    for f in range(2):
                        nc.tensor.matmul(
                            op[:], hT_layers[l][f][:, 128 * b:128 * (b + 1)],
                            linw_sb[:, 2 * l + f, :],
                            start=(kk == 0), stop=(kk == 5))
                        kk += 1
                ob = workp.tile([128, D], mybir.dt.float32, tag="ob",
                                name=f"ob_{b}")
                if b_zero:
                    nc.vector.tensor_copy(ob[:], op[:])
                else:
                    nc.vector.tensor_tensor(ob[:], op[:], linb_sb[:],
                                            mybir.AluOpType.add)
                nc.sync.dma_start(out_d[128 * b:128 * (b + 1), :], ob[:])
            consume_post(L - 1, 1, prev[0], prev[1])

            # ---- JK final linear, B half
            for b in range(NTA, NT):
                op = psB.tile([128, D], mybir.dt.float32, tag="gemm")
                kk = 0
                for l in range(L):
                    for f in range(2):
                        nc.tensor.matmul(
                            op[:], hT_layers[l][f][:, 128 * b:128 * (b + 1)],
                            linw_sb[:, 2 * l + f, :],
                            start=(kk == 0), stop=(kk == 5))
                        kk += 1
                ob = workp.tile([128, D], mybir.dt.float32, tag="ob")
                if b_zero:
                    nc.vector.tensor_copy(ob[:], op[:])
                else:
                    nc.vector.tensor_tensor(ob[:], op[:], linb_sb[:],
                                            mybir.AluOpType.add)
                nc.sync.dma_start(out_d[128 * b:128 * (b + 1), :], ob[:])
    nc.compile()
    return nc


# ------------------------------------------------------------------- runner
def _build_args(pre):
    return (pre["Wp"], pre["c"], pre["lin_w"], pre["lin_b"])


def _in_maps(pre):
    sel8 = pre["sel"].astype(np.float32).astype(F8)
    return [
        {"xT": np.ascontiguousarray(pre["xT"][cc].astype(F8) if False
                                     else pre["xT"][cc].astype(BF16)),
         "sel": np.ascontiguousarray(
             sel8[cc].reshape(128, GT * PAIRS * 2 * 128)),
         "dinv": np.ascontiguousarray(pre["dinv_t"][cc])}
        for cc in range(NCORES)
    ]


def _postprocess(pre, results):
    out = np.empty((N, D), np.float32)
    owner, local = pre["owner"], pre["local"]
    for cc in range(NCORES):
        m = owner == cc
        out[m] = results[cc]["out"][local[m]]
    return out


def kernel(x, edge_index, Ws, bs, bn_scale, bn_bias, bn_mean, bn_var,
           lin_w, lin_b):
    pre = _preprocess(x, edge_index, Ws, bs, bn_scale, bn_bias, bn_mean,
                      bn_var, lin_w, lin_b)
    nc = _build(*_build_args(pre))
    in_maps = _in_maps(pre)
    kw = {}
    if _TRACE:
        kw = dict(trace=True)
    res = run_bass_kernel_spmd(nc, in_maps, core_ids=list(range(NCORES)), **kw)
    kernel.last_results = res
    return _postprocess(pre, res.results)
